# revision 12
# baseline (speedup 1.0000x reference)
"""Chamfer loss kernel for Trainium2 (8 NeuronCores).

Problem: pred [4, 8192, 3], target [4, 8192, 3] ->
    scalar = mean_b( mean_n min_m dist(pred_bn, target_bm)
           + mean_m min_n dist(pred_bn, target_bm) )

Strategy (v2: TensorEngine distance matrix + native DVE reductions)
------------------------------------------------------------------
* 8 cores = 4 batches x 2 pred-halves.  Core (2b+h) owns pred rows
  [h*4096, (h+1)*4096) of batch b and all 8192 targets.
* The [4096, 8192] squared-distance matrix is produced on the
  TensorEngine as ONE K=11 fp16 matmul per output tile:
      d2[i,j] = -2 p_i.t_j + |t_j|^2 + |p_i|^2
  with every f32 operand split hi/lo into two fp16 values
  (x = xh + xl, products exact in the f32 PSUM accumulate), giving
  ~1e-6 final relative error (validated vs reference in numpy):
      K rows:  (ah,th) (ah,tl) (al,th)  x3 coords   [9 rows]
               (1, tnh) (1, tnl)                    [2 rows]
  |p|^2 enters later, for free, as the per-partition bias of the
  ScalarEngine PSUM->SBUF pass.
* Dataflow per 128-pred row tile (32 tiles):
    PE:  4 chunks x 4 matmuls (N=512, fp16) -> PSUM f32 [128, 2048]
    ACT: Relu(psum + pn_bias) -> SBUF fp16 row buffer  (clamps the
         tiny negative cancellation glitches, so Sqrt never NaNs)
    DVE: colacc = min(colacc, rowbuf)            [fp16 2x mode]
         tensor_tensor_reduce: fold rowbuf halves with min AND
         min-reduce the fold -> per-row min in one pass [fp16 2x]
  d1 = sqrt(row mins) directly.
* d2 (min over preds = over partitions): PE transposes colacc in
  [128,128] blocks (vs identity) into PSUM, DVE min-reduces each
  group of 4 blocks -> [128, 64]; ACT sqrt.  (DVE lanes cannot read
  across partitions; gpsimd partition reduce is ms-slow.)
* Host side only shards inputs and averages the tiny per-core min
  vectors (pure gather/unshard arithmetic).
"""

import numpy as np

_P = 128
_N = 4096          # pred points per core
_M = 8192          # target points
_NRT = _N // _P    # 32 pred row tiles
_CH = 2048         # psum chunk columns (4 banks)
_NCH = _M // _CH   # 4 chunks per row tile
_NBLK = _M // _P   # 64 transpose blocks for d2


def _build_kernel(repeats=1, no_transpose=False, no_ttr=False, no_tnmm=False):
    import concourse.bacc as bacc
    import concourse.mybir as mybir
    import concourse.tile as tile

    f32 = mybir.dt.float32
    f16 = mybir.dt.float16
    AF = mybir.ActivationFunctionType
    ALU = mybir.AluOpType
    AX = mybir.AxisListType

    nc = bacc.Bacc("TRN2", target_bir_lowering=False, debug=False, num_devices=8)
    p_d = nc.dram_tensor("pxyz", [3, _N], f32, kind="ExternalInput")
    t_d = nc.dram_tensor("txyz", [3, _M], f32, kind="ExternalInput")
    psc_d = nc.dram_tensor("psc", [_P, 3, _NRT], f32, kind="ExternalInput")
    eye_d = nc.dram_tensor("eye", [_P, _P], f16, kind="ExternalInput")
    d1_d = nc.dram_tensor("d1", [_P, _NRT], f32, kind="ExternalOutput")
    d2_d = nc.dram_tensor("d2", [_P, _NBLK], f32, kind="ExternalOutput")

    with tile.TileContext(nc) as tc:
        with (
            tc.tile_pool(name="const", bufs=1) as cpool,
            tc.tile_pool(name="work", bufs=2) as wpool,
            tc.tile_pool(name="psum", bufs=2, space="PSUM") as ppool,
        ):
            p32 = cpool.tile([3, _N], f32)
            t32 = cpool.tile([3, _M], f32)
            psc = cpool.tile([_P, 3, _NRT], f32)
            eye = cpool.tile([_P, _P], f16)
            ones31 = cpool.tile([3, 1], f32)
            ones_s = cpool.tile([1, _N], f16)
            d1 = cpool.tile([_P, _NRT], f32)
            d2 = cpool.tile([_P, _NBLK], f32)
            nc.sync.dma_start(p32[:], p_d[:])
            nc.sync.dma_start(t32[:], t_d[:])
            nc.sync.dma_start(psc[:], psc_d[:])
            nc.sync.dma_start(eye[:], eye_d[:])
            nc.vector.memset(ones31[:], 1.0)
            nc.vector.memset(ones_s[:], 1.0)

            for _ in range(repeats):
                # K=11 packed matmul operands.
                # Pk rows: 0-2 ah=fp16(-2p), 3-5 ah dup, 6-8 al, 9-10 ones
                # Tk rows: 0-2 th=fp16(t),  3-5 tl,     6-8 th dup, 9 tnh, 10 tnl
                Pk = wpool.tile([11, _N], f16, tag="Pk")
                Tk = wpool.tile([11, _M], f16, tag="Tk")
                nc.sync.dma_start(Pk[9:10, :], ones_s[:])
                nc.sync.dma_start(Pk[10:11, :], ones_s[:])

                # ---- target-side coord splits ----
                nc.scalar.activation(Tk[0:3, :], t32[:], AF.Copy)        # th
                tl_s = wpool.tile([3, _M], f16, tag="tl_s", bufs=1)
                nc.vector.tensor_tensor(tl_s[:], t32[:], Tk[0:3, :], ALU.subtract)
                nc.sync.dma_start(Tk[3:6, :], tl_s[:])
                nc.sync.dma_start(Tk[6:9, :], Tk[0:3, :])

                # ---- pred-side coord splits ----
                nc.scalar.activation(Pk[0:3, :], p32[:], AF.Copy, scale=-2.0)  # ah
                al_s = wpool.tile([3, _N], f16, tag="al_s", bufs=1)
                nc.vector.scalar_tensor_tensor(
                    al_s[:], p32[:], -2.0, Pk[0:3, :], op0=ALU.mult, op1=ALU.subtract
                )
                nc.sync.dma_start(Pk[3:6, :], Pk[0:3, :])
                nc.sync.dma_start(Pk[6:9, :], al_s[:])

                # ---- tn = |t|^2 via tiny f32 matmul (partition sum) ----
                for c in range([0, _NCH][not no_tnmm]):
                    sl = slice(c * _CH, (c + 1) * _CH)
                    t2c = wpool.tile([3, _CH], f32, tag="t2c", bufs=1)
                    nc.scalar.activation(t2c[:], t32[:, sl], AF.Square)
                    tn_ps = ppool.tile([1, _CH], f32, tag="ps")
                    for q in range(_CH // 512):
                        nc.tensor.matmul(
                            tn_ps[:, q * 512:(q + 1) * 512],
                            ones31[:],
                            t2c[:, q * 512:(q + 1) * 512],
                            start=True, stop=True,
                        )
                    tnh_s = wpool.tile([1, _CH], f16, tag="tnh_s", bufs=1)
                    nc.scalar.activation(tnh_s[:], tn_ps[:], AF.Copy)
                    tnl_s = wpool.tile([1, _CH], f16, tag="tnl_s", bufs=1)
                    nc.vector.tensor_tensor(tnl_s[:], tn_ps[:], tnh_s[:], ALU.subtract)
                    nc.sync.dma_start(Tk[9:10, sl], tnh_s[:])
                    nc.sync.dma_start(Tk[10:11, sl], tnl_s[:])

                # ---- pn = |p|^2 per (partition, row tile) for the ACT bias ----
                psq = wpool.tile([_P, 3, _NRT], f32, tag="psq")
                nc.scalar.activation(psq[:], psc[:], AF.Square)
                pn = wpool.tile([_P, _NRT], f32, tag="pn")
                nc.vector.tensor_tensor(pn[:], psq[:, 0, :], psq[:, 1, :], ALU.add)
                nc.vector.tensor_tensor(pn[:], pn[:], psq[:, 2, :], ALU.add)

                part = wpool.tile([_P, _NRT], f32, tag="part")
                colacc = wpool.tile([_P, _M], f16, tag="colacc", bufs=1)

                # ---- main loop over pred row tiles ----
                for r in range(_NRT):
                    if r == 0:
                        rb = colacc
                    else:
                        rb = wpool.tile([_P, _M], f16, tag="rowbuf")
                    lhsT = Pk[:, r * _P:(r + 1) * _P]
                    for c in range(_NCH):
                        ps = ppool.tile([_P, _CH], f32, tag="ps")
                        for q in range(_CH // 512):
                            nc.tensor.matmul(
                                ps[:, q * 512:(q + 1) * 512],
                                lhsT,
                                Tk[:, c * _CH + q * 512:c * _CH + (q + 1) * 512],
                                start=True, stop=True,
                            )
                        nc.scalar.activation(
                            rb[:, c * _CH:(c + 1) * _CH], ps[:],
                            AF.Relu, bias=pn[:, r:r + 1], scale=1.0,
                        )
                    if r > 0:
                        nc.vector.tensor_tensor(colacc[:], colacc[:], rb[:], ALU.min)
                    # row-min: binary fold at fp16 2x rate, short 1x reduce.
                    # (tensor_tensor_reduce would fuse this but crashes the HW)
                    scr = wpool.tile([_P, _M // 2], f16, tag="scr", bufs=1)
                    nc.vector.tensor_tensor(
                        scr[:], rb[:, 0:_M // 2], rb[:, _M // 2:_M], ALU.min
                    )
                    w = _M // 4
                    while w >= 512:
                        nc.vector.tensor_tensor(
                            scr[:, 0:w], scr[:, 0:w], scr[:, w:2 * w], ALU.min
                        )
                        w //= 2
                    nc.vector.tensor_reduce(
                        part[:, r:r + 1], scr[:, 0:512], axis=AX.X, op=ALU.min
                    )

                # ---- d2: min over partitions via PE transpose blocks ----
                d2sq = wpool.tile([_P, _NBLK], f32, tag="d2sq")
                if no_transpose:
                    nc.vector.memset(d2sq[:], 1.0)
                for g in range([_NBLK // 4, 0][no_transpose]):
                    tp = ppool.tile([_P, 4, _P], f16, tag="ps")
                    for q in range(4):
                        blk = 4 * g + q
                        nc.tensor.transpose(
                            tp[:, q, :], colacc[:, blk * _P:(blk + 1) * _P], eye[:]
                        )
                    nc.vector.tensor_reduce(
                        d2sq[:, 4 * g:4 * (g + 1)], tp[:], axis=AX.X, op=ALU.min
                    )

                nc.scalar.activation(d1[:], part[:], AF.Sqrt)
                nc.scalar.activation(d2[:], d2sq[:], AF.Sqrt)

            nc.sync.dma_start(d1_d[:], d1[:])
            nc.sync.dma_start(d2_d[:], d2[:])

    nc.compile()
    return nc


_NC_CACHE = None
_LAST_RESULT = None  # BassKernelResults of the most recent run (for test harness)


def _get_nc():
    global _NC_CACHE
    if _NC_CACHE is None:
        _NC_CACHE = _build_kernel()
    return _NC_CACHE


def _make_in_maps(pred, target):
    """Per-core input dict list: core 2b+h = batch b, pred half h."""
    B = pred.shape[0]
    half = pred.shape[1] // 2
    eye = np.eye(_P, dtype=np.float16)
    in_maps = []
    for b in range(B):
        txyz = np.ascontiguousarray(target[b].astype(np.float32).T)
        for h in range(2):
            ph = pred[b, h * half:(h + 1) * half].astype(np.float32)
            in_maps.append({
                "pxyz": np.ascontiguousarray(ph.T),
                "txyz": txyz,
                "psc": np.ascontiguousarray(
                    ph.reshape(_NRT, _P, 3).transpose(1, 2, 0)
                ),
                "eye": eye,
            })
    return in_maps


def kernel(pred, target):
    from concourse.bass_utils import run_bass_kernel_spmd

    pred = np.asarray(pred, dtype=np.float32)
    target = np.asarray(target, dtype=np.float32)
    B = pred.shape[0]

    in_maps = _make_in_maps(pred, target)
    nc = _get_nc()
    res = run_bass_kernel_spmd(nc, in_maps, list(range(2 * B)))
    global _LAST_RESULT
    _LAST_RESULT = res

    total = 0.0
    for b in range(B):
        d1a = res.results[2 * b]["d1"]          # [128, 32] dist1, pred rows 0..4095
        d1b = res.results[2 * b + 1]["d1"]      # [128, 32] dist1, pred rows 4096..
        # d2[p, blk] = min dist for target blk*128+p (partial: that half's preds)
        d2a = res.results[2 * b]["d2"].T.ravel()      # [8192]
        d2b = res.results[2 * b + 1]["d2"].T.ravel()
        ch1 = 0.5 * (d1a.mean(dtype=np.float64) + d1b.mean(dtype=np.float64))
        ch2 = np.minimum(d2a, d2b).mean(dtype=np.float64)
        total += ch1 + ch2
    return np.float32(total / B)


# revision 13
# speedup vs baseline: 9.0440x; 9.0440x over previous
"""Chamfer loss kernel for Trainium2 (8 NeuronCores).

Problem: pred [4, 8192, 3], target [4, 8192, 3] ->
    scalar = mean_b( mean_n min_m dist(pred_bn, target_bm)
           + mean_m min_n dist(pred_bn, target_bm) )

Strategy (v3: instruction-count-minimal, page-reset-scan distance op)
---------------------------------------------------------------------
The execution backend is bound by per-instruction dispatch (~60-100us
per instruction, nearly independent of element count), so the kernel is
organized around the fewest, largest instructions possible.  The
architectural floor is one instruction per 128-pred row tile (the
partition width): 32 distance instructions per core.

* 8 cores = 4 batches x 2 pred-halves.  Core (2b+h) owns pred rows
  [h*4096, (h+1)*4096) of batch b and all 8192 targets.
* ONE custom DVE instruction per row tile computes all of
  s[p, j] = -dist^2(pred_p, target_j) over [128 x 8192 x 3coords]:
  the element stream is pages of 3 (the xyz coords of one (pred,
  target) pair); the op's body is a page-RESET scan
      acc = sum_page( -(t_c[j] - p_c[p])^2 )
  (a `ResetScan` node: the steady state is a normal running sum, and
  the SUB_DIM_DONE page-boundary step state re-seeds the accumulator
  from the current element -- a ~10-line extension of the stock
  dve_spec lowering, verified on hardware against numpy).
  Inputs need no data movement: in0 is a [P, 8192, 3] strided view of
  the broadcast target coords, in1 a stride-0 replay view of this
  tile's pred coords.  The output AP is stride-0 along the page dim,
  so consecutive page elements overwrite and only the page-final value
  (the complete -dist^2) lands: D[p, j] directly, no 3x intermediate.
* Distances accumulate into an 8-slot SBUF slab; per chunk of <=7
  tiles, ONE native tensor_reduce(max, axis=X) yields the per-pred
  row maxima of s (= -min dist^2, the d1 direction) and ONE strided
  tensor_reduce(max) over [P, 8192, slots] folds the column direction
  into a running colmax slab slot (the d2 direction).
* One gpsimd partition_all_reduce(max) finishes d2 across partitions;
  two ACT Sqrt(scale=-1) instructions produce distances.
  Total: 32 + 5 + 5 + 1 + 2 = 45 instructions per pass (vs ~105 for
  the previous kernel), every reduction in native max form.
* fp16 coords / fp16 slab, fp32 in-op accumulation: numpy-validated
  end-to-end error ~2e-5 (tolerance 2e-2).
* Host side only shards inputs, converts dtypes, and averages the tiny
  per-core min-distance vectors (pure gather/unshard arithmetic).
"""

import dataclasses

import numpy as np

_P = 128
_N = 4096          # pred points per core
_M = 8192          # target points
_NRT = _N // _P    # 32 pred row tiles
# chunk sizes over the 8-slot D slab (slot 7 = column-max accumulator):
_CHUNKS = [4, 7, 7, 7, 7]


def _install_resetscan():
    """Extend the custom-DVE lowering with a page-reset scan node."""
    import concourse.dve_spec as dve_spec
    from concourse.dve_spec import Scan, AluOp

    rs = getattr(dve_spec, "_ANT_RESETSCAN", None)
    if rs is not None:
        return rs

    @dataclasses.dataclass(frozen=True)
    class ResetScan(Scan):
        """Scan that re-seeds from the current element at each page
        boundary: steady: acc = op(acc, expr); boundary elem: acc = expr."""
        pass

    orig = dve_spec._scan_overrides

    def patched(scans, node_stage):
        seed, step = orig(scans, node_stage)
        for scan in scans:
            if isinstance(scan, ResetScan):
                step[node_stage[scan]] = dve_spec._Stage(AluOp.BYPASS, scan.expr)
        return seed, step

    dve_spec._scan_overrides = patched
    dve_spec._ANT_RESETSCAN = ResetScan
    return ResetScan


def _register_op():
    """Register the negated-distance page-reset op (idempotent)."""
    import concourse.dve_ops as dve_ops
    from concourse.dve_uop import DveOpSpec
    from concourse.dve_spec import Spec, Src0, Src1, Zero, sq, lower, AluOp

    name = "CH_NSQ3_PR_ANT"
    for op in dve_ops.OPS:
        if op.name == name:
            return op

    ResetScan = _install_resetscan()
    body = ResetScan(AluOp.ADD, Zero - sq(Src0 - Src1))

    def _ref(in0, in1, c0, c1, c2):
        d = in0.astype(np.float32) - in1.astype(np.float32)
        return np.cumsum(-(d * d), axis=-1)  # page-reset scan along pages

    spec = Spec(body=body, reference=_ref)
    op = dve_ops.DveOp(name, spec, subdim=True, uops_sha={})
    dve_ops.OPS.append(op)
    row = dve_ops._CUSTOM_DVE_ROW_BASE + len(dve_ops.OPS) - 1
    assert row < 0x20, "custom DVE opcode row overflow"
    dve_ops._SUB_OPCODE_FOR_NAME[name] = row
    dve_ops.CUSTOM_DVE_SPECS[name] = spec
    for ver in ("v3", "v4"):
        s = DveOpSpec(name=name, opcode=row, uops=lower(spec, ver=ver),
                      rd1_en=dve_ops.has_src1(spec))
        op.uops_sha[ver] = s.sha(ver)
    return op


def _build_kernel(repeats=1):
    import concourse.bacc as bacc
    import concourse.bass as bass
    import concourse.bass_isa as bass_isa
    import concourse.mybir as mybir
    import concourse.tile as tile

    f32 = mybir.dt.float32
    f16 = mybir.dt.float16
    AF = mybir.ActivationFunctionType
    ALU = mybir.AluOpType
    AX = mybir.AxisListType
    op = _register_op()

    nc = bacc.Bacc("TRN2", target_bir_lowering=False, debug=False, num_devices=8)
    t_d = nc.dram_tensor("txyz", [3, _M], f16, kind="ExternalInput")
    psc_d = nc.dram_tensor("psc", [_P, 3, _NRT], f16, kind="ExternalInput")
    d1_d = nc.dram_tensor("d1", [_P, _NRT], f32, kind="ExternalOutput")
    d2_d = nc.dram_tensor("d2", [1, _M], f16, kind="ExternalOutput")

    with tile.TileContext(nc) as tc:
        with tc.tile_pool(name="pool", bufs=1) as pool:
            t3h = pool.tile([_P, 3, _M], f16)
            psc = pool.tile([_P, 3, _NRT], f16)
            D = pool.tile([_P, 8, _M], f16)     # slots 0-6 tiles, 7 = colmax
            part = pool.tile([_P, _NRT], f32)   # per-pred max of s = -min d^2
            d1 = pool.tile([_P, _NRT], f32)
            d2 = pool.tile([1, _M], f16)

            # input staging: broadcast target coords to all partitions
            nc.sync.dma_start(
                t3h[:], bass.AP(t_d, 0, [[0, _P], [_M, 3], [1, _M]])
            )
            nc.sync.dma_start(psc[:], psc_d[:])

            in0 = t3h[:].rearrange("p c j -> p j c")  # [P, M, 3] strided view

            for _ in range(repeats):
                base = 0
                for n in _CHUNKS:
                    for i in range(n):
                        r = base + i
                        in1 = (psc[:, :, r].unsqueeze(1)
                               .broadcast_to((_P, _M, 3)))
                        out = (D[:, i, :].unsqueeze(2)
                               .broadcast_to((_P, _M, 3)))
                        nc.vector._custom_dve(op, out=out, in0=in0, in1=in1)
                    # d1 direction: per-tile row max of s
                    nc.vector.tensor_reduce(
                        part[:, base:base + n], D[:, 0:n, :],
                        axis=AX.X, op=ALU.max,
                    )
                    # d2 direction: fold the fresh slots (and, after the
                    # first chunk, the running colmax in slot 7) into slot 7
                    nsl = n if base == 0 else 8
                    nc.vector.tensor_reduce(
                        D[:, 7, :],
                        D[:, 0:nsl, :].rearrange("p s j -> p j s"),
                        axis=AX.X, op=ALU.max,
                    )
                    base += n

                # d2: max over partitions, then sqrt(-x)
                nc.gpsimd.partition_all_reduce(
                    D[:, 0, :], D[:, 7, :], _P, bass_isa.ReduceOp.max
                )
                nc.scalar.activation(d2[:], D[0:1, 0, :], AF.Sqrt, scale=-1.0)
                nc.scalar.activation(d1[:], part[:], AF.Sqrt, scale=-1.0)

            nc.sync.dma_start(d1_d[:], d1[:])
            nc.sync.dma_start(d2_d[:], d2[:])

    nc.compile()
    return nc


_NC_CACHE = None
_LAST_RESULT = None


def _get_nc():
    global _NC_CACHE
    if _NC_CACHE is None:
        _NC_CACHE = _build_kernel()
    return _NC_CACHE


def _make_in_maps(pred, target):
    """Per-core input dict list: core 2b+h = batch b, pred half h."""
    B = pred.shape[0]
    half = pred.shape[1] // 2
    in_maps = []
    for b in range(B):
        txyz = np.ascontiguousarray(target[b].T.astype(np.float16))
        for h in range(2):
            ph = pred[b, h * half:(h + 1) * half]
            in_maps.append({
                "txyz": txyz,
                "psc": np.ascontiguousarray(
                    ph.reshape(_NRT, _P, 3).transpose(1, 2, 0)
                ).astype(np.float16),
            })
    return in_maps


def kernel(pred, target):
    from concourse.bass_utils import run_bass_kernel_spmd

    pred = np.asarray(pred, dtype=np.float32)
    target = np.asarray(target, dtype=np.float32)
    B = pred.shape[0]

    in_maps = _make_in_maps(pred, target)
    nc = _get_nc()
    res = run_bass_kernel_spmd(nc, in_maps, list(range(2 * B)))
    global _LAST_RESULT
    _LAST_RESULT = res

    total = 0.0
    for b in range(B):
        d1a = res.results[2 * b]["d1"]        # [128, 32] dist1, pred rows 0..4095
        d1b = res.results[2 * b + 1]["d1"]    # [128, 32] dist1, pred rows 4096..
        d2a = res.results[2 * b]["d2"][0].astype(np.float32)    # [8192] partial
        d2b = res.results[2 * b + 1]["d2"][0].astype(np.float32)
        ch1 = 0.5 * (d1a.mean(dtype=np.float64) + d1b.mean(dtype=np.float64))
        ch2 = np.minimum(d2a, d2b).mean(dtype=np.float64)
        total += ch1 + ch2
    return np.float32(total / B)


# revision 14
# speedup vs baseline: 16.9210x; 1.8710x over previous
"""Chamfer loss kernel for Trainium2 (8 NeuronCores).

Problem: pred [4, 8192, 3], target [4, 8192, 3] ->
    scalar = mean_b( mean_n min_m dist(pred_bn, target_bm)
           + mean_m min_n dist(pred_bn, target_bm) )

Strategy (v3: instruction-count-minimal, page-reset-scan distance op)
---------------------------------------------------------------------
The execution backend is bound by per-instruction dispatch (~60-100us
per instruction, nearly independent of element count), so the kernel is
organized around the fewest, largest instructions possible.  The
architectural floor is one instruction per 128-pred row tile (the
partition width): 32 distance instructions per core.

* 8 cores = 4 batches x 2 pred-halves.  Core (2b+h) owns pred rows
  [h*4096, (h+1)*4096) of batch b and all 8192 targets.
* ONE custom DVE instruction per row tile computes all of
  s[p, j] = -dist^2(pred_p, target_j) over [128 x 8192 x 3coords]:
  the element stream is pages of 3 (the xyz coords of one (pred,
  target) pair); the op's body is a page-RESET scan
      acc = sum_page( -(t_c[j] - p_c[p])^2 )
  (a `ResetScan` node: the steady state is a normal running sum, and
  the SUB_DIM_DONE page-boundary step state re-seeds the accumulator
  from the current element -- a ~10-line extension of the stock
  dve_spec lowering, verified on hardware against numpy).
  Inputs need no data movement: in0 is a [P, 8192, 3] strided view of
  the broadcast target coords, in1 a stride-0 replay view of this
  tile's pred coords.  The output AP is stride-0 along the page dim,
  so consecutive page elements overwrite and only the page-final value
  (the complete -dist^2) lands: D[p, j] directly, no 3x intermediate.
* Distances accumulate into an 8-slot SBUF slab; per chunk of <=7
  tiles, ONE native tensor_reduce(max, axis=X) yields the per-pred
  row maxima of s (= -min dist^2, the d1 direction) and ONE strided
  tensor_reduce(max) over [P, 8192, slots] folds the column direction
  into a running colmax slab slot (the d2 direction).
* One gpsimd partition_all_reduce(max) finishes d2 across partitions;
  two ACT Sqrt(scale=-1) instructions produce distances.
  Total: 32 + 5 + 5 + 1 + 2 = 45 instructions per pass (vs ~105 for
  the previous kernel), every reduction in native max form.
* fp16 coords / fp16 slab, fp32 in-op accumulation: numpy-validated
  end-to-end error ~2e-5 (tolerance 2e-2).
* Host side only shards inputs, converts dtypes, and averages the tiny
  per-core min-distance vectors (pure gather/unshard arithmetic).
"""

import dataclasses

import numpy as np

_P = 128
_N = 4096          # pred points per core
_M = 8192          # target points
_NRT = _N // _P    # 32 pred row tiles
# chunk sizes over the 8-slot D slab (slot 7 = column-max accumulator):
_CHUNKS = [4, 7, 7, 7, 7]


def _install_resetscan():
    """Extend the custom-DVE lowering with a page-reset scan node."""
    import concourse.dve_spec as dve_spec
    from concourse.dve_spec import Scan, AluOp

    rs = getattr(dve_spec, "_ANT_RESETSCAN", None)
    if rs is not None:
        return rs

    @dataclasses.dataclass(frozen=True)
    class ResetScan(Scan):
        """Scan that re-seeds from the current element at each page
        boundary: steady: acc = op(acc, expr); boundary elem: acc = expr."""
        pass

    orig = dve_spec._scan_overrides

    def patched(scans, node_stage):
        seed, step = orig(scans, node_stage)
        for scan in scans:
            if isinstance(scan, ResetScan):
                step[node_stage[scan]] = dve_spec._Stage(AluOp.BYPASS, scan.expr)
        return seed, step

    dve_spec._scan_overrides = patched
    dve_spec._ANT_RESETSCAN = ResetScan
    return ResetScan


def _register_op():
    """Register the negated-distance page-reset op (idempotent)."""
    import concourse.dve_ops as dve_ops
    from concourse.dve_uop import DveOpSpec
    from concourse.dve_spec import Spec, Src0, Src1, Zero, sq, lower, AluOp

    name = "CH_NSQ3_PR_ANT"
    for op in dve_ops.OPS:
        if op.name == name:
            return op

    ResetScan = _install_resetscan()
    body = ResetScan(AluOp.ADD, Zero - sq(Src0 - Src1))

    def _ref(in0, in1, c0, c1, c2):
        d = in0.astype(np.float32) - in1.astype(np.float32)
        return np.cumsum(-(d * d), axis=-1)  # page-reset scan along pages

    spec = Spec(body=body, reference=_ref)
    op = dve_ops.DveOp(name, spec, subdim=True, uops_sha={})
    dve_ops.OPS.append(op)
    row = dve_ops._CUSTOM_DVE_ROW_BASE + len(dve_ops.OPS) - 1
    assert row < 0x20, "custom DVE opcode row overflow"
    dve_ops._SUB_OPCODE_FOR_NAME[name] = row
    dve_ops.CUSTOM_DVE_SPECS[name] = spec
    for ver in ("v3", "v4"):
        s = DveOpSpec(name=name, opcode=row, uops=lower(spec, ver=ver),
                      rd1_en=dve_ops.has_src1(spec))
        op.uops_sha[ver] = s.sha(ver)
    return op


def _build_kernel(repeats=1):
    import concourse.bacc as bacc
    import concourse.bass as bass
    import concourse.bass_isa as bass_isa
    import concourse.mybir as mybir
    import concourse.tile as tile

    f32 = mybir.dt.float32
    f16 = mybir.dt.float16
    AF = mybir.ActivationFunctionType
    ALU = mybir.AluOpType
    AX = mybir.AxisListType
    op = _register_op()

    nc = bacc.Bacc("TRN2", target_bir_lowering=False, debug=False, num_devices=8)
    t_d = nc.dram_tensor("txyz", [_M, 3], f16, kind="ExternalInput")
    psc_d = nc.dram_tensor("psc", [_P, 3, _NRT], f16, kind="ExternalInput")
    d1_d = nc.dram_tensor("d1", [_P, _NRT], f32, kind="ExternalOutput")
    d2_d = nc.dram_tensor("d2", [1, _M], f16, kind="ExternalOutput")

    with tile.TileContext(nc) as tc:
        with tc.tile_pool(name="pool", bufs=1) as pool:
            t3i = pool.tile([_P, _M, 3], f16)   # c-interleaved: contiguous in0
            psc = pool.tile([_P, 3, _NRT], f16)
            D = pool.tile([_P, 8, _M], f16)     # slots 0-6 tiles, 7 = colmax
            part = pool.tile([_P, _NRT], f32)   # per-pred max of s = -min d^2
            d1 = pool.tile([_P, _NRT], f32)
            d2 = pool.tile([1, _M], f16)

            # input staging: broadcast interleaved target coords (flat copy
            # replicated to all partitions; matches the natural [M, 3] host
            # layout, so the custom op's in0 is fully contiguous)
            nc.sync.dma_start(
                t3i[:], bass.AP(t_d, 0, [[0, _P], [1, _M * 3]])
            )
            nc.sync.dma_start(psc[:], psc_d[:])

            in0 = t3i[:]

            for _ in range(repeats):
                base = 0
                for n in _CHUNKS:
                    for i in range(n):
                        r = base + i
                        in1 = (psc[:, :, r].unsqueeze(1)
                               .broadcast_to((_P, _M, 3)))
                        out = (D[:, i, :].unsqueeze(2)
                               .broadcast_to((_P, _M, 3)))
                        nc.vector._custom_dve(op, out=out, in0=in0, in1=in1)
                    # d1 direction: per-tile row max of s
                    nc.vector.tensor_reduce(
                        part[:, base:base + n], D[:, 0:n, :],
                        axis=AX.X, op=ALU.max,
                    )
                    # d2 direction: fold the fresh slots (and, after the
                    # first chunk, the running colmax in slot 7) into slot 7
                    nsl = n if base == 0 else 8
                    nc.vector.tensor_reduce(
                        D[:, 7, :],
                        D[:, 0:nsl, :].rearrange("p s j -> p j s"),
                        axis=AX.X, op=ALU.max,
                    )
                    base += n

                # d2: max over partitions, then sqrt(-x)
                nc.gpsimd.partition_all_reduce(
                    D[:, 0, :], D[:, 7, :], _P, bass_isa.ReduceOp.max
                )
                nc.scalar.activation(d2[:], D[0:1, 0, :], AF.Sqrt, scale=-1.0)
                nc.scalar.activation(d1[:], part[:], AF.Sqrt, scale=-1.0)

            nc.sync.dma_start(d1_d[:], d1[:])
            nc.sync.dma_start(d2_d[:], d2[:])

    nc.compile()
    return nc


_NC_CACHE = None
_LAST_RESULT = None


def _get_nc():
    global _NC_CACHE
    if _NC_CACHE is None:
        _NC_CACHE = _build_kernel()
    return _NC_CACHE


def _make_in_maps(pred, target):
    """Per-core input dict list: core 2b+h = batch b, pred half h."""
    B = pred.shape[0]
    half = pred.shape[1] // 2
    in_maps = []
    for b in range(B):
        txyz = np.ascontiguousarray(target[b].astype(np.float16))
        for h in range(2):
            ph = pred[b, h * half:(h + 1) * half]
            in_maps.append({
                "txyz": txyz,
                "psc": np.ascontiguousarray(
                    ph.reshape(_NRT, _P, 3).transpose(1, 2, 0)
                ).astype(np.float16),
            })
    return in_maps


def kernel(pred, target):
    from concourse.bass_utils import run_bass_kernel_spmd

    pred = np.asarray(pred, dtype=np.float32)
    target = np.asarray(target, dtype=np.float32)
    B = pred.shape[0]

    in_maps = _make_in_maps(pred, target)
    nc = _get_nc()
    res = run_bass_kernel_spmd(nc, in_maps, list(range(2 * B)))
    global _LAST_RESULT
    _LAST_RESULT = res

    total = 0.0
    for b in range(B):
        d1a = res.results[2 * b]["d1"]        # [128, 32] dist1, pred rows 0..4095
        d1b = res.results[2 * b + 1]["d1"]    # [128, 32] dist1, pred rows 4096..
        d2a = res.results[2 * b]["d2"][0].astype(np.float32)    # [8192] partial
        d2b = res.results[2 * b + 1]["d2"][0].astype(np.float32)
        ch1 = 0.5 * (d1a.mean(dtype=np.float64) + d1b.mean(dtype=np.float64))
        ch2 = np.minimum(d2a, d2b).mean(dtype=np.float64)
        total += ch1 + ch2
    return np.float32(total / B)


# revision 16
# speedup vs baseline: 18.6938x; 1.1048x over previous
"""Chamfer loss kernel for Trainium2 (8 NeuronCores).

Problem: pred [4, 8192, 3], target [4, 8192, 3] ->
    scalar = mean_b( mean_n min_m dist(pred_bn, target_bm)
           + mean_m min_n dist(pred_bn, target_bm) )

Strategy (v3: instruction-count-minimal, page-reset-scan distance op)
---------------------------------------------------------------------
The execution backend is bound by per-instruction dispatch (~60-100us
per instruction, nearly independent of element count), so the kernel is
organized around the fewest, largest instructions possible.  The
architectural floor is one instruction per 128-pred row tile (the
partition width): 32 distance instructions per core.

* 8 cores = 4 batches x 2 pred-halves.  Core (2b+h) owns pred rows
  [h*4096, (h+1)*4096) of batch b and all 8192 targets.
* ONE custom DVE instruction per row tile computes all of
  s[p, j] = -dist^2(pred_p, target_j) over [128 x 8192 x 3coords]:
  the element stream is pages of 3 (the xyz coords of one (pred,
  target) pair); the op's body is a page-RESET scan
      acc = sum_page( -(t_c[j] - p_c[p])^2 )
  (a `ResetScan` node: the steady state is a normal running sum, and
  the SUB_DIM_DONE page-boundary step state re-seeds the accumulator
  from the current element -- a ~10-line extension of the stock
  dve_spec lowering, verified on hardware against numpy).
  Inputs need no data movement: in0 is a [P, 8192, 3] strided view of
  the broadcast target coords, in1 a stride-0 replay view of this
  tile's pred coords.  The output AP is stride-0 along the page dim,
  so consecutive page elements overwrite and only the page-final value
  (the complete -dist^2) lands: D[p, j] directly, no 3x intermediate.
* Distances accumulate into an 8-slot SBUF slab; per chunk of <=7
  tiles, ONE native tensor_reduce(max, axis=X) yields the per-pred
  row maxima of s (= -min dist^2, the d1 direction) and ONE strided
  tensor_reduce(max) over [P, 8192, slots] folds the column direction
  into a running colmax slab slot (the d2 direction).
* One gpsimd partition_all_reduce(max) finishes d2 across partitions;
  two ACT Sqrt(scale=-1) instructions produce distances.
  Total: 32 + 5 + 5 + 1 + 2 = 45 instructions per pass (vs ~105 for
  the previous kernel), every reduction in native max form.
* fp16 coords / fp16 slab, fp32 in-op accumulation: numpy-validated
  end-to-end error ~2e-5 (tolerance 2e-2).
* Host side only shards inputs, converts dtypes, and averages the tiny
  per-core min-distance vectors (pure gather/unshard arithmetic).
"""

import dataclasses

import numpy as np

_P = 128
_N = 4096          # pred points per core
_M = 8192          # target points
_NRT = _N // _P    # 32 pred row tiles
# chunk sizes over the 9-slot D slab (slot 8 = column-max accumulator):
_CHUNKS = [8, 8, 8, 8]


def _install_resetscan():
    """Extend the custom-DVE lowering with a page-reset scan node."""
    import concourse.dve_spec as dve_spec
    from concourse.dve_spec import Scan, AluOp

    rs = getattr(dve_spec, "_ANT_RESETSCAN", None)
    if rs is not None:
        return rs

    @dataclasses.dataclass(frozen=True)
    class ResetScan(Scan):
        """Scan that re-seeds from the current element at each page
        boundary: steady: acc = op(acc, expr); boundary elem: acc = expr."""
        pass

    orig = dve_spec._scan_overrides

    def patched(scans, node_stage):
        seed, step = orig(scans, node_stage)
        for scan in scans:
            if isinstance(scan, ResetScan):
                step[node_stage[scan]] = dve_spec._Stage(AluOp.BYPASS, scan.expr)
        return seed, step

    dve_spec._scan_overrides = patched
    dve_spec._ANT_RESETSCAN = ResetScan
    return ResetScan


def _register_op():
    """Register the negated-distance page-reset op (idempotent)."""
    import concourse.dve_ops as dve_ops
    from concourse.dve_uop import DveOpSpec
    from concourse.dve_spec import Spec, Src0, Src1, Zero, sq, lower, AluOp

    name = "CH_NSQ3_PR_ANT"
    for op in dve_ops.OPS:
        if op.name == name:
            return op

    ResetScan = _install_resetscan()
    body = ResetScan(AluOp.ADD, Zero - sq(Src0 - Src1))

    def _ref(in0, in1, c0, c1, c2):
        d = in0.astype(np.float32) - in1.astype(np.float32)
        return np.cumsum(-(d * d), axis=-1)  # page-reset scan along pages

    spec = Spec(body=body, reference=_ref)
    op = dve_ops.DveOp(name, spec, subdim=True, uops_sha={})
    dve_ops.OPS.append(op)
    row = dve_ops._CUSTOM_DVE_ROW_BASE + len(dve_ops.OPS) - 1
    assert row < 0x20, "custom DVE opcode row overflow"
    dve_ops._SUB_OPCODE_FOR_NAME[name] = row
    dve_ops.CUSTOM_DVE_SPECS[name] = spec
    for ver in ("v3", "v4"):
        s = DveOpSpec(name=name, opcode=row, uops=lower(spec, ver=ver),
                      rd1_en=dve_ops.has_src1(spec))
        op.uops_sha[ver] = s.sha(ver)
    return op


def _build_kernel(repeats=1):
    import concourse.bacc as bacc
    import concourse.bass as bass
    import concourse.bass_isa as bass_isa
    import concourse.mybir as mybir
    import concourse.tile as tile

    f32 = mybir.dt.float32
    f16 = mybir.dt.float16
    AF = mybir.ActivationFunctionType
    ALU = mybir.AluOpType
    AX = mybir.AxisListType
    op = _register_op()

    nc = bacc.Bacc("TRN2", target_bir_lowering=False, debug=False, num_devices=8)
    t_d = nc.dram_tensor("txyz", [_M, 3], f16, kind="ExternalInput")
    psc_d = nc.dram_tensor("psc", [_P, 3, _NRT], f16, kind="ExternalInput")
    d1_d = nc.dram_tensor("d1", [_P, _NRT], f32, kind="ExternalOutput")
    d2_d = nc.dram_tensor("d2", [1, _M], f16, kind="ExternalOutput")

    with tile.TileContext(nc) as tc:
        with tc.tile_pool(name="pool", bufs=1) as pool:
            t3i = pool.tile([_P, _M, 3], f16)   # c-interleaved: contiguous in0
            psc = pool.tile([_P, 3, _NRT], f16)
            D = pool.tile([_P, 9, _M], f16)     # slots 0-7 tiles, 8 = colmax
            part = pool.tile([_P, _NRT], f32)   # per-pred max of s = -min d^2
            d1 = pool.tile([_P, _NRT], f32)

            # input staging: broadcast interleaved target coords (flat copy
            # replicated to all partitions; matches the natural [M, 3] host
            # layout, so the custom op's in0 is fully contiguous)
            nc.sync.dma_start(
                t3i[:], bass.AP(t_d, 0, [[0, _P], [1, _M * 3]])
            )
            nc.sync.dma_start(psc[:], psc_d[:])

            in0 = t3i[:]

            for _ in range(repeats):
                base = 0
                for n in _CHUNKS:
                    for i in range(n):
                        r = base + i
                        in1 = (psc[:, :, r].unsqueeze(1)
                               .broadcast_to((_P, _M, 3)))
                        out = (D[:, i, :].unsqueeze(2)
                               .broadcast_to((_P, _M, 3)))
                        nc.vector._custom_dve(op, out=out, in0=in0, in1=in1)
                    # d1 direction: per-tile row max of s
                    nc.vector.tensor_reduce(
                        part[:, base:base + n], D[:, 0:n, :],
                        axis=AX.X, op=ALU.max,
                    )
                    # d2 direction: fold the fresh slots (and, after the
                    # first chunk, the running colmax in slot 7) into slot 7
                    nsl = n if base == 0 else 9
                    nc.vector.tensor_reduce(
                        D[:, 8, :],
                        D[:, 0:nsl, :].rearrange("p s j -> p j s"),
                        axis=AX.X, op=ALU.max,
                    )
                    base += n

                # d2: max over partitions, then sqrt(-x)
                nc.gpsimd.partition_all_reduce(
                    D[:, 0, :], D[:, 8, :], _P, bass_isa.ReduceOp.max
                )
                nc.scalar.activation(D[0:1, 1, :], D[0:1, 0, :], AF.Sqrt,
                                     scale=-1.0)
                nc.scalar.activation(d1[:], part[:], AF.Sqrt, scale=-1.0)

            nc.sync.dma_start(d1_d[:], d1[:])
            nc.sync.dma_start(d2_d[:], D[0:1, 1, :])

    nc.compile()
    return nc


_NC_CACHE = None
_LAST_RESULT = None


def _get_nc():
    global _NC_CACHE
    if _NC_CACHE is None:
        _NC_CACHE = _build_kernel()
    return _NC_CACHE


def _make_in_maps(pred, target):
    """Per-core input dict list: core 2b+h = batch b, pred half h."""
    B = pred.shape[0]
    half = pred.shape[1] // 2
    in_maps = []
    for b in range(B):
        txyz = np.ascontiguousarray(target[b].astype(np.float16))
        for h in range(2):
            ph = pred[b, h * half:(h + 1) * half]
            in_maps.append({
                "txyz": txyz,
                "psc": np.ascontiguousarray(
                    ph.reshape(_NRT, _P, 3).transpose(1, 2, 0)
                ).astype(np.float16),
            })
    return in_maps


def kernel(pred, target):
    from concourse.bass_utils import run_bass_kernel_spmd

    pred = np.asarray(pred, dtype=np.float32)
    target = np.asarray(target, dtype=np.float32)
    B = pred.shape[0]

    in_maps = _make_in_maps(pred, target)
    nc = _get_nc()
    res = run_bass_kernel_spmd(nc, in_maps, list(range(2 * B)))
    global _LAST_RESULT
    _LAST_RESULT = res

    total = 0.0
    for b in range(B):
        d1a = res.results[2 * b]["d1"]        # [128, 32] dist1, pred rows 0..4095
        d1b = res.results[2 * b + 1]["d1"]    # [128, 32] dist1, pred rows 4096..
        d2a = res.results[2 * b]["d2"][0].astype(np.float32)    # [8192] partial
        d2b = res.results[2 * b + 1]["d2"][0].astype(np.float32)
        ch1 = 0.5 * (d1a.mean(dtype=np.float64) + d1b.mean(dtype=np.float64))
        ch2 = np.minimum(d2a, d2b).mean(dtype=np.float64)
        total += ch1 + ch2
    return np.float32(total / B)
